# revision 1
# baseline (speedup 1.0000x reference)
"""Identity kernel for nn_InvWaveletTransformLayer (64, 1048576) f32.

The reference op is the identity (pywt.waverec with a length-1 coeffs list
returns cA unchanged), so the kernel is a pure memory copy. We shard the
batch axis (64 rows) across 8 NeuronCores (8 rows = 32 MiB per core) and
issue a single large DRAM->DRAM DMA per core.
"""

import numpy as np

import concourse.bass as bass
import concourse.mybir as mybir
from concourse.bass_utils import run_bass_kernel_spmd

BATCH = 64
SIG_LEN = 1 << 20
N_CORES = 8
ROWS = BATCH // N_CORES  # 8 rows (32 MiB) per core

_NC_CACHE = None


def _build_nc() -> bass.Bass:
    global _NC_CACHE
    if _NC_CACHE is not None:
        return _NC_CACHE

    nc = bass.Bass()
    x = nc.declare_dram_parameter("x", [ROWS, SIG_LEN], mybir.dt.float32, isOutput=False)
    out = nc.declare_dram_parameter("out", [ROWS, SIG_LEN], mybir.dt.float32, isOutput=True)

    with nc.Block() as block, nc.semaphore("dma_sem") as dma_sem:

        @block.sync
        def _(sync: bass.BassEngine):
            sync.dma_start(out=out[:], in_=x[:]).then_inc(dma_sem, 16)
            sync.wait_ge(dma_sem, 16)

    _NC_CACHE = nc
    return nc


def kernel(x: np.ndarray) -> np.ndarray:
    nc = _build_nc()
    in_maps = [{"x": x[c * ROWS : (c + 1) * ROWS]} for c in range(N_CORES)]
    res = run_bass_kernel_spmd(nc, in_maps, list(range(N_CORES))).results
    return np.concatenate([r["out"] for r in res], axis=0)


# revision 3
# speedup vs baseline: 1.0252x; 1.0252x over previous
"""Identity kernel for nn_InvWaveletTransformLayer (64, 1048576) f32.

The reference op is the identity (pywt.waverec with a length-1 coeffs list
returns cA unchanged), so the kernel is a pure memory copy. We shard the
batch axis (64 rows) across 8 NeuronCores (8 rows = 32 MiB per core) and
issue a single large DRAM->DRAM DMA per core.
"""

import numpy as np

import concourse.bass as bass
import concourse.mybir as mybir
from concourse.bass_utils import run_bass_kernel_spmd

BATCH = 64
SIG_LEN = 1 << 20
N_CORES = 8
ROWS = BATCH // N_CORES  # 8 rows (32 MiB) per core

_NC_CACHE = None


def _build_nc() -> bass.Bass:
    global _NC_CACHE
    if _NC_CACHE is not None:
        return _NC_CACHE

    nc = bass.Bass()
    x = nc.declare_dram_parameter("x", [ROWS, SIG_LEN], mybir.dt.float32, isOutput=False)
    out = nc.declare_dram_parameter("out", [ROWS, SIG_LEN], mybir.dt.float32, isOutput=True)

    # SWDGE (gpsimd) ring: same HBM-wall body time as HWDGE, but measured
    # slightly better max-core distribution across paired reps.
    with nc.Block() as block, nc.semaphore("dma_sem") as dma_sem:

        @block.gpsimd
        def _(g: bass.BassEngine):
            g.dma_start(out=out[:], in_=x[:]).then_inc(dma_sem, 16)
            g.wait_ge(dma_sem, 16)

    _NC_CACHE = nc
    return nc


def kernel(x: np.ndarray) -> np.ndarray:
    x = np.ascontiguousarray(np.asarray(x), dtype=np.float32)
    nc = _build_nc()
    in_maps = [{"x": x[c * ROWS : (c + 1) * ROWS]} for c in range(N_CORES)]
    res = run_bass_kernel_spmd(nc, in_maps, list(range(N_CORES))).results
    return np.concatenate([r["out"] for r in res], axis=0)
